# revision 7
# baseline (speedup 1.0000x reference)
"""Trainium2 Bass kernel for a dense transformer block.

Reference computation (per batch item, fp32 inputs):
    h   = LN(x; ln1_g, ln1_b)
    q,k,v = per-head projections of h        (H=8 heads, D=64)
    scores = (q @ k^T) * C**-0.5, causal-masked, softmax
    o   = scores @ v, heads concatenated
    x2  = x + o @ w_proj + b_proj
    out = x2 + relu(LN(x2; ln2_g, ln2_b) @ w1 + b1) @ w2 + b2

Sharding: pure data parallel over batch. B=32 across 8 cores -> 4 batch
items per core, weights replicated, no collectives.

Per-core design notes:
  - LN affine transforms are folded into the following matmul weights on
    the host: wq/wk/wv absorb diag(ln1_g) (and wq also the C**-0.5 score
    scale); w1 absorbs diag(ln2_g) and b1 absorbs ln2_b @ w1. When the
    LN bias is nonzero for the QKV path, rank-1 correction matmuls are
    emitted (skipped for zero bias).
  - LN stats run in [t, c] layout (free-dim bn_stats); the normalized
    bf16 output is transposed to [c, t] by DMA-xbar transposes, feeding
    all matmuls with contraction over c. rstd = Exp(-0.5*Log(var+eps))
    so the whole kernel uses one ACT table set (exp/log/relu/copy).
  - Scores are computed transposed per head: scoresT[s, t] with
    lhsT=k^T slice / rhs=q^T (head-pair packed [128, T]); exp'd scores
    (bf16) are then directly the lhsT of the attn@v matmuls. The causal
    mask multiplies the diagonal 128x128 block after exp (DVE, bf16).
  - v is stored interleaved [128, 8, 65] with a ones column per head, so
    each attn@v matmul (N=65) also produces the softmax denominator in
    its last column; two heads share one PSUM bank [128, 130]. The
    normalize step is one fused tensor_tensor multiply with a step-0
    broadcast AP of the reciprocal denominators.
  - FFN hidden z is computed transposed [f, t] so relu+b1 is one ACT op
    with per-partition bias and z^T directly feeds FFN2 as lhsT.
  - x2 (attention residual) is spilled to a DRAM scratch tensor and
    reloaded for the final residual add to keep SBUF under budget.
  - Phase order: LN1 for all batch items up front (overlaps the initial
    weight DMA); per-item QKV->scores/exp->attn@v->proj->LN2; FFN for
    all items at the end. This gives the PE filler work across batch
    items during LN dependency chains.

All matmuls run in bf16 (fp32 PSUM accumulation).
"""

import contextlib

import numpy as np
import ml_dtypes

import concourse.bass as bass
import concourse.bacc as bacc
import concourse.tile as tile
import concourse.mybir as mybir
from concourse import bass_utils

B, T, C, H, D = 32, 512, 512, 8, 64
NCORES = 8
NB = B // NCORES          # batch items per core
P = 128
NT = T // P               # 4 token tiles
NCT = C // P              # 4 channel tiles
FF = 4 * C                # 2048
NF = FF // P              # 16 hidden tiles
EPS = 1e-5
SCALE = float(C) ** -0.5
NPAIR = H // 2            # head pairs (2 heads x 64 = 128 partitions)
DA = D + 1                # v columns per head incl. ones column

F32 = mybir.dt.float32
BF16 = mybir.dt.bfloat16
AF = mybir.ActivationFunctionType
OP = mybir.AluOpType
bf16 = ml_dtypes.bfloat16

_CACHE = {}


def _bcast_free(ap, reps):
    """Append a step-0 innermost dim: each free element read `reps` times."""
    return bass.AP(tensor=ap.tensor, offset=ap.offset, ap=[*ap.ap, [0, reps]])


def _body(tc, io, cfg):
    nc = tc.nc
    (x_d, wq_d, wk_d, wv_d, wp_d, w1_d, w2_d, b1_d, bp_d, b2_d, cq_d,
     trimask_d, ones_row_d, x2s_d, out_d) = io

    ctx = contextlib.ExitStack()
    with ctx:
        singles = ctx.enter_context(tc.tile_pool(name="singles", bufs=1))
        xp = ctx.enter_context(tc.tile_pool(name="xp", bufs=6))
        x2p = ctx.enter_context(tc.tile_pool(name="x2p", bufs=6))
        x2rp = ctx.enter_context(tc.tile_pool(name="x2rp", bufs=4))
        nrm = ctx.enter_context(tc.tile_pool(name="nrm", bufs=6))
        stat = ctx.enter_context(tc.tile_pool(name="stat", bufs=12))
        hTp = ctx.enter_context(tc.tile_pool(name="hTp", bufs=NB * NCT))
        qkp = ctx.enter_context(tc.tile_pool(name="qkp", bufs=2 * NPAIR + 2))
        vp = ctx.enter_context(tc.tile_pool(name="vp", bufs=NT + 2))
        expp = ctx.enter_context(tc.tile_pool(name="expp", bufs=H + 2))
        osp = ctx.enter_context(tc.tile_pool(name="osp", bufs=NT + 1))
        oTp = ctx.enter_context(tc.tile_pool(name="oTp", bufs=NCT + 1))
        h2Tp = ctx.enter_context(tc.tile_pool(name="h2Tp", bufs=NB * NCT))
        zp = ctx.enter_context(tc.tile_pool(name="zp", bufs=NF + 1))
        outp = ctx.enter_context(tc.tile_pool(name="outp", bufs=4))
        # PSUM: 8 banks total
        mmp = ctx.enter_context(tc.tile_pool(name="mmp", bufs=3, space="PSUM"))
        scp = ctx.enter_context(tc.tile_pool(name="scp", bufs=3, space="PSUM"))
        opp = ctx.enter_context(tc.tile_pool(name="opp", bufs=2, space="PSUM"))

        def load(pool, dram_ap, dtype):
            t = pool.tile(list(dram_ap.shape), dtype, tag=dram_ap.tensor.name)
            nc.sync.dma_start(out=t, in_=dram_ap)
            return t

        wq_sb = load(singles, wq_d, BF16)    # [128, NCT, 512]  (c, kt, h*64+d)
        wk_sb = load(singles, wk_d, BF16)
        wv_sb = load(singles, wv_d, BF16)
        wp_sb = load(singles, wp_d, BF16)    # [128, NCT, 512]
        w1_sb = load(singles, w1_d, BF16)    # [128, NCT, 2048]
        w2_sb = load(singles, w2_d, BF16)    # [128, NF, 512]
        b1_sb = load(singles, b1_d, F32)     # [128, NF]
        bp_sb = load(singles, bp_d, BF16) if cfg["has_bp"] else None
        b2_sb = load(singles, b2_d, BF16) if cfg["has_b2"] else None
        cq_sb = load(singles, cq_d, BF16) if cfg["has_ln1b"] else None  # [3,512]
        trimask = load(singles, trimask_d, BF16)    # [128,128] keep t>=s
        ones_row = load(singles, ones_row_d, BF16)  # [1, 512]
        eps_t = singles.tile([P, 1], F32)
        nc.vector.memset(eps_t, EPS)

        def layernorm_T(x_tiles, hT_pool, n_tag):
            """x tiles [t,c] fp32 -> (x-mu)*rstd bf16 -> DMA-transpose to
            [c,t] tiles. Affine is folded into downstream weights."""
            n_tiles = []
            for t in range(NT):
                st6 = stat.tile([P, 6], F32, tag="st6")
                nc.vector.bn_stats(out=st6, in_=x_tiles[t])
                mv = stat.tile([P, 2], F32, tag="mv")
                nc.vector.bn_aggr(out=mv, in_=st6)
                lnv = stat.tile([P, 1], F32, tag="lnv")
                nc.scalar.activation(out=lnv, in_=mv[:, 1:2], func=AF.Ln,
                                     bias=eps_t)
                rstd = stat.tile([P, 1], F32, tag="rstd")
                nc.scalar.activation(out=rstd, in_=lnv, func=AF.Exp,
                                     scale=-0.5)
                n_t = nrm.tile([P, T], BF16, tag=n_tag)
                nc.vector.tensor_scalar(out=n_t, in0=x_tiles[t],
                                        scalar1=mv[:, 0:1], scalar2=rstd,
                                        op0=OP.subtract, op1=OP.mult)
                n_tiles.append(n_t)
            hT = []
            for i in range(NCT):
                h_i = hT_pool.tile([P, T], BF16)
                for t in range(NT):
                    nc.sync.dma_start_transpose(
                        out=h_i[:, P * t:P * (t + 1)],
                        in_=n_tiles[t][:, P * i:P * (i + 1)])
                hT.append(h_i)
            return hT

        # ---- LN1 for all batch items (overlaps initial weight DMA) ----
        hTs = []
        for b in range(NB):
            x_tiles = []
            for t in range(NT):
                x_t = xp.tile([P, C], F32)
                nc.sync.dma_start(out=x_t, in_=x_d[b, P * t:P * (t + 1), :])
                x_tiles.append(x_t)
            hTs.append(layernorm_T(x_tiles, hTp, "n1"))

        h2Ts = []
        for b in range(NB):
            hT = hTs[b]
            # ---- QKV ----
            qT, kT = [], []
            for pr in range(NPAIR):
                sl = slice(P * pr, P * (pr + 1))
                qps = mmp.tile([P, T], F32, tag="mm")
                for kt in range(NCT):
                    nc.tensor.matmul(qps, wq_sb[:, kt, sl], hT[kt],
                                     start=(kt == 0),
                                     stop=(kt == NCT - 1
                                           and not cfg["has_ln1b"]),
                                     skip_group_check=True)
                if cfg["has_ln1b"]:
                    nc.tensor.matmul(qps, cq_sb[0:1, sl], ones_row,
                                     start=False, stop=True,
                                     skip_group_check=True)
                q_sb = qkp.tile([P, T], BF16, tag="qk")
                nc.scalar.activation(out=q_sb, in_=qps, func=AF.Copy)
                qT.append(q_sb)
                kps = mmp.tile([P, T], F32, tag="mm")
                for kt in range(NCT):
                    nc.tensor.matmul(kps, wk_sb[:, kt, sl], hT[kt],
                                     start=(kt == 0),
                                     stop=(kt == NCT - 1
                                           and not cfg["has_ln1b"]),
                                     skip_group_check=True)
                if cfg["has_ln1b"]:
                    nc.tensor.matmul(kps, cq_sb[1:2, sl], ones_row,
                                     start=False, stop=True,
                                     skip_group_check=True)
                k_sb = qkp.tile([P, T], BF16, tag="qk")
                nc.vector.tensor_copy(out=k_sb, in_=kps)
                kT.append(k_sb)
            v_aug = []
            for st in range(NT):
                sl = slice(P * st, P * (st + 1))
                vps = mmp.tile([P, C], F32, tag="mm")
                for kt in range(NCT):
                    nc.tensor.matmul(vps, hT[kt][:, sl], wv_sb[:, kt, :],
                                     start=(kt == 0),
                                     stop=(kt == NCT - 1
                                           and not cfg["has_ln1b"]),
                                     skip_group_check=True)
                if cfg["has_ln1b"]:
                    nc.tensor.matmul(vps, ones_row[:, 0:P], cq_sb[2:3, :],
                                     start=False, stop=True,
                                     skip_group_check=True)
                va = vp.tile([P, H, DA], BF16)
                nc.vector.memset(va[:, :, D:DA], 1.0)
                nc.vector.tensor_copy(
                    out=va[:, :, 0:D],
                    in_=vps[:].rearrange("p (h d) -> p h d", h=H))
                v_aug.append(va)

            # ---- scores^T + exp (per head, per s-tile) ----
            # expT[h][i] covers t in [P*i, T): tile [P, T - P*i]
            expT = [[None] * NT for _ in range(H)]
            for h in range(H):
                pr, off = divmod(h, 2)
                off *= D
                for i in range(NT):
                    w = T - P * i
                    sc = scp.tile([P, T], F32, tag="sc")
                    nc.tensor.matmul(sc[:, 0:w],
                                     kT[pr][off:off + D, P * i:P * (i + 1)],
                                     qT[pr][off:off + D, P * i:],
                                     start=True, stop=True)
                    e_t = expp.tile([P, w], BF16, tag=f"e{i}")
                    nc.scalar.activation(out=e_t, in_=sc[:, 0:w], func=AF.Exp)
                    # causal mask on the diagonal block (keep t >= s)
                    nc.vector.tensor_tensor(out=e_t[:, 0:P], in0=e_t[:, 0:P],
                                            in1=trimask, op=OP.mult)
                    expT[h][i] = e_t

            # ---- attention out + normalize (t-tile major, head pairs) ----
            o_sb = []
            for m in range(NT):
                o_t = osp.tile([P, C], BF16)
                for pr in range(NPAIR):
                    o2 = opp.tile([P, 2 * DA], F32, tag="op")
                    for h01 in range(2):
                        h = 2 * pr + h01
                        for i in range(m + 1):
                            lhs = expT[h][i][:, P * (m - i):P * (m - i + 1)]
                            nc.tensor.matmul(o2[:, DA * h01:DA * (h01 + 1)],
                                             lhs, v_aug[i][:, h, :],
                                             start=(i == 0), stop=(i == m),
                                             skip_group_check=True)
                    l_ap = bass.AP(tensor=o2[:].tensor,
                                   offset=o2[:, D:D + 1].offset,
                                   ap=[o2[:].ap[0], [DA, 2]])
                    linv = stat.tile([P, 2], F32, tag="linv")
                    nc.vector.reciprocal(out=linv, in_=l_ap)
                    o_part = bass.AP(tensor=o2[:].tensor, offset=o2[:].offset,
                                     ap=[o2[:].ap[0], [DA, 2], [1, D]])
                    out3 = o_t[:, P * pr:P * (pr + 1)].rearrange(
                        "p (a d) -> p a d", a=2)
                    nc.vector.tensor_tensor(out=out3, in0=o_part,
                                            in1=_bcast_free(linv[:], D),
                                            op=OP.mult)
                o_sb.append(o_t)

            # ---- transpose o (DMA xbar) ----
            oT = []
            for i in range(NCT):
                oT_i = oTp.tile([P, T], BF16)
                for m in range(NT):
                    nc.sync.dma_start_transpose(
                        out=oT_i[:, P * m:P * (m + 1)],
                        in_=o_sb[m][:, P * i:P * (i + 1)])
                oT.append(oT_i)

            # ---- proj + residual; spill x2 to DRAM scratch ----
            x2_tiles = []
            for m in range(NT):
                yps = mmp.tile([P, C], F32, tag="mm")
                for kt in range(NCT):
                    nc.tensor.matmul(yps, oT[kt][:, P * m:P * (m + 1)],
                                     wp_sb[:, kt, :], start=(kt == 0),
                                     stop=(kt == NCT - 1
                                           and not cfg["has_bp"]),
                                     skip_group_check=True)
                if cfg["has_bp"]:
                    nc.tensor.matmul(yps, ones_row[:, 0:P], bp_sb,
                                     start=False, stop=True,
                                     skip_group_check=True)
                x_t = xp.tile([P, C], F32)
                nc.sync.dma_start(out=x_t, in_=x_d[b, P * m:P * (m + 1), :])
                x2_t = x2p.tile([P, C], F32)
                nc.vector.tensor_tensor(out=x2_t, in0=yps, in1=x_t, op=OP.add)
                nc.sync.dma_start(out=x2s_d[b, P * m:P * (m + 1), :], in_=x2_t)
                x2_tiles.append(x2_t)

            # ---- LN2 (affine folded into w1/b1) ----
            h2Ts.append(layernorm_T(x2_tiles, h2Tp, "n2"))

        # ---- FFN for all batch items ----
        for b in range(NB):
            h2T = h2Ts[b]
            zT = []
            for j in range(NF):
                zps = mmp.tile([P, T], F32, tag="mm")
                for kt in range(NCT):
                    nc.tensor.matmul(zps, w1_sb[:, kt, P * j:P * (j + 1)],
                                     h2T[kt], start=(kt == 0),
                                     stop=(kt == NCT - 1))
                z_j = zp.tile([P, T], BF16)
                nc.scalar.activation(out=z_j, in_=zps, func=AF.Relu,
                                     bias=b1_sb[:, j:j + 1])
                zT.append(z_j)
            for m in range(NT):
                fps = mmp.tile([P, C], F32, tag="mm")
                for kt in range(NF):
                    nc.tensor.matmul(fps, zT[kt][:, P * m:P * (m + 1)],
                                     w2_sb[:, kt, :], start=(kt == 0),
                                     stop=(kt == NF - 1
                                           and not cfg["has_b2"]),
                                     skip_group_check=True)
                if cfg["has_b2"]:
                    nc.tensor.matmul(fps, ones_row[:, 0:P], b2_sb,
                                     start=False, stop=True,
                                     skip_group_check=True)
                x2r = x2rp.tile([P, C], F32)
                nc.sync.dma_start(out=x2r,
                                  in_=x2s_d[b, P * m:P * (m + 1), :])
                o_t = outp.tile([P, C], F32)
                nc.vector.tensor_tensor(out=o_t, in0=fps, in1=x2r, op=OP.add)
                nc.sync.dma_start(out=out_d[b, P * m:P * (m + 1), :], in_=o_t)


def _build(cfg):
    nc = bacc.Bacc("TRN2", target_bir_lowering=False, debug=False,
                   num_devices=NCORES)
    d = nc.dram_tensor
    io = (
        d("x", [NB, T, C], F32, kind="ExternalInput").ap(),
        d("wq", [P, NCT, C], BF16, kind="ExternalInput").ap(),
        d("wk", [P, NCT, C], BF16, kind="ExternalInput").ap(),
        d("wv", [P, NCT, C], BF16, kind="ExternalInput").ap(),
        d("wp", [P, NCT, C], BF16, kind="ExternalInput").ap(),
        d("w1", [P, NCT, FF], BF16, kind="ExternalInput").ap(),
        d("w2", [P, NF, C], BF16, kind="ExternalInput").ap(),
        d("b1", [P, NF], F32, kind="ExternalInput").ap(),
        d("bp", [1, C], BF16, kind="ExternalInput").ap(),
        d("b2", [1, C], BF16, kind="ExternalInput").ap(),
        d("cq", [3, C], BF16, kind="ExternalInput").ap(),
        d("trimask", [P, P], BF16, kind="ExternalInput").ap(),
        d("ones_row", [1, C], BF16, kind="ExternalInput").ap(),
        d("x2s", [NB, T, C], F32, kind="Internal").ap(),
        d("out", [NB, T, C], F32, kind="ExternalOutput").ap(),
    )
    with tile.TileContext(nc) as tc:
        _body(tc, io, cfg)
    nc.compile()
    return nc


def _ktile(w, part):
    """[K, M] -> [128, K//128, M] with K = 128*kt + p."""
    k, m = w.shape
    return np.ascontiguousarray(
        w.reshape(k // part, part, m).transpose(1, 0, 2))


def _col(v, part):
    """[N] -> [128, N//128] with n = 128*j + p."""
    return np.ascontiguousarray(v.reshape(-1, part).T)


def kernel(**inputs):
    f32 = lambda a: np.asarray(a, np.float32)
    x = f32(inputs["x"])
    wq = f32(inputs["wq"]).transpose(1, 0, 2).reshape(C, C)   # [c, h*D+d]
    wk = f32(inputs["wk"]).transpose(1, 0, 2).reshape(C, C)
    wv = f32(inputs["wv"]).transpose(1, 0, 2).reshape(C, C)
    w1 = f32(inputs["w1"])
    g1 = f32(inputs["ln1_g"])[:, None]
    b1ln = f32(inputs["ln1_b"])
    g2 = f32(inputs["ln2_g"])[:, None]
    b2ln = f32(inputs["ln2_b"])

    cfg = {
        "has_ln1b": bool(np.any(b1ln != 0.0)),
        "has_bp": bool(np.any(f32(inputs["b_proj"]) != 0.0)),
        "has_b2": bool(np.any(f32(inputs["b2"]) != 0.0)),
    }
    key = tuple(sorted(cfg.items()))
    if key not in _CACHE:
        _CACHE[key] = _build(cfg)
    nc = _CACHE[key]

    # fold LN affines (and the score scale) into the weights
    wq_f = (g1 * wq) * SCALE
    wk_f = g1 * wk
    wv_f = g1 * wv
    w1_f = g2 * w1
    b1_f = b2ln @ w1 + f32(inputs["b1"])
    cq = np.stack([(b1ln @ wq) * SCALE, b1ln @ wk, b1ln @ wv])  # [3, C]

    common = {
        "wq": _ktile(wq_f, P).astype(bf16),
        "wk": _ktile(wk_f, P).astype(bf16),
        "wv": _ktile(wv_f, P).astype(bf16),
        "wp": _ktile(f32(inputs["w_proj"]), P).astype(bf16),
        "w1": _ktile(w1_f, P).astype(bf16),
        "w2": _ktile(f32(inputs["w2"]), P).astype(bf16),
        "b1": _col(b1_f, P),
        "bp": f32(inputs["b_proj"]).reshape(1, C).astype(bf16),
        "b2": f32(inputs["b2"]).reshape(1, C).astype(bf16),
        "cq": cq.astype(bf16),
        "trimask": np.triu(np.ones((P, P), np.float32)).astype(bf16),
        "ones_row": np.ones((1, C), bf16),
    }
    in_maps = [dict(common, x=np.ascontiguousarray(x[c * NB:(c + 1) * NB]))
               for c in range(NCORES)]

    res = bass_utils.run_bass_kernel_spmd(nc, in_maps,
                                          core_ids=list(range(NCORES)),
                                          trace=_CACHE.get("trace", False))
    _CACHE["last_result"] = res
    return np.concatenate([r["out"] for r in res.results], axis=0)


# revision 8
# speedup vs baseline: 1.3950x; 1.3950x over previous
"""Trainium2 Bass kernel for a dense transformer block.

Reference computation (per batch item, fp32 inputs):
    h   = LN(x; ln1_g, ln1_b)
    q,k,v = per-head projections of h        (H=8 heads, D=64)
    scores = (q @ k^T) * C**-0.5, causal-masked, softmax
    o   = scores @ v, heads concatenated
    x2  = x + o @ w_proj + b_proj
    out = x2 + relu(LN(x2; ln2_g, ln2_b) @ w1 + b1) @ w2 + b2

Sharding: pure data parallel over batch. B=32 across 8 cores -> 4 batch
items per core, weights replicated, no collectives.

Per-core design notes:
  - LN affine transforms are folded into the following matmul weights on
    the host: wq/wk/wv absorb diag(ln1_g) (and wq also the C**-0.5 score
    scale); w1 absorbs diag(ln2_g) and b1 absorbs ln2_b @ w1. When the
    LN bias is nonzero for the QKV path, rank-1 correction matmuls are
    emitted (skipped for zero bias).
  - LN stats run in [t, c] layout (free-dim bn_stats); the normalized
    bf16 output is transposed to [c, t] by DMA-xbar transposes, feeding
    all matmuls with contraction over c. rstd = Exp(-0.5*Log(var+eps))
    so the whole kernel uses one ACT table set (exp/log/relu/copy).
  - Scores are computed transposed per head: scoresT[s, t] with
    lhsT=k^T slice / rhs=q^T (head-pair packed [128, T]); exp'd scores
    (bf16) are then directly the lhsT of the attn@v matmuls. The causal
    mask multiplies the diagonal 128x128 block after exp (DVE, bf16).
  - v is stored interleaved [128, 8, 65] with a ones column per head, so
    each attn@v matmul (N=65) also produces the softmax denominator in
    its last column; two heads share one PSUM bank [128, 130]. The
    normalize step is one fused tensor_tensor multiply with a step-0
    broadcast AP of the reciprocal denominators.
  - FFN hidden z is computed transposed [f, t] so relu+b1 is one ACT op
    with per-partition bias and z^T directly feeds FFN2 as lhsT.
  - x2 (attention residual) is spilled to a DRAM scratch tensor and
    reloaded for the final residual add to keep SBUF under budget.
  - Phase order: LN1 for all batch items up front (overlaps the initial
    weight DMA); per-item QKV->scores/exp->attn@v->proj->LN2; FFN for
    all items at the end. This gives the PE filler work across batch
    items during LN dependency chains.

All matmuls run in bf16 (fp32 PSUM accumulation).
"""

import contextlib

import numpy as np
import ml_dtypes

import concourse.bass as bass
import concourse.bacc as bacc
import concourse.tile as tile
import concourse.mybir as mybir
from concourse import bass_utils

B, T, C, H, D = 32, 512, 512, 8, 64
NCORES = 8
NB = B // NCORES          # batch items per core
P = 128
NT = T // P               # 4 token tiles
NCT = C // P              # 4 channel tiles
FF = 4 * C                # 2048
NF = FF // P              # 16 hidden tiles
EPS = 1e-5
SCALE = float(C) ** -0.5
NPAIR = H // 2            # head pairs (2 heads x 64 = 128 partitions)
DA = D + 1                # v columns per head incl. ones column

F32 = mybir.dt.float32
BF16 = mybir.dt.bfloat16
AF = mybir.ActivationFunctionType
OP = mybir.AluOpType
bf16 = ml_dtypes.bfloat16

_CACHE = {}


def _bcast_free(ap, reps):
    """Append a step-0 innermost dim: each free element read `reps` times."""
    return bass.AP(tensor=ap.tensor, offset=ap.offset, ap=[*ap.ap, [0, reps]])


def _body(tc, io, cfg):
    nc = tc.nc
    (x_d, wq_d, wk_d, wv_d, wp_d, w1_d, w2_d, b1_d, bp_d, b2_d, cq_d,
     trimask_d, ident_d, ones_row_d, x2s_d, out_d) = io

    ctx = contextlib.ExitStack()
    with ctx:
        singles = ctx.enter_context(tc.tile_pool(name="singles", bufs=1))
        xp = ctx.enter_context(tc.tile_pool(name="xp", bufs=6))
        x2p = ctx.enter_context(tc.tile_pool(name="x2p", bufs=6))
        x2rp = ctx.enter_context(tc.tile_pool(name="x2rp", bufs=4))
        nrm = ctx.enter_context(tc.tile_pool(name="nrm", bufs=6))
        stat = ctx.enter_context(tc.tile_pool(name="stat", bufs=12))
        hTp = ctx.enter_context(tc.tile_pool(name="hTp", bufs=NB * NCT))
        qkp = ctx.enter_context(tc.tile_pool(name="qkp", bufs=2 * NPAIR + 2))
        vp = ctx.enter_context(tc.tile_pool(name="vp", bufs=NT + 2))
        expp = ctx.enter_context(tc.tile_pool(name="expp", bufs=H + 2))
        osp = ctx.enter_context(tc.tile_pool(name="osp", bufs=NT + 1))
        oTp = ctx.enter_context(tc.tile_pool(name="oTp", bufs=NCT + 1))
        h2Tp = ctx.enter_context(tc.tile_pool(name="h2Tp", bufs=NB * NCT))
        zp = ctx.enter_context(tc.tile_pool(name="zp", bufs=NF + 1))
        outp = ctx.enter_context(tc.tile_pool(name="outp", bufs=4))
        # PSUM: 8 banks total
        mmp = ctx.enter_context(tc.tile_pool(name="mmp", bufs=3, space="PSUM"))
        tpp = ctx.enter_context(tc.tile_pool(name="tpp", bufs=1, space="PSUM"))
        scp = ctx.enter_context(tc.tile_pool(name="scp", bufs=2, space="PSUM"))
        opp = ctx.enter_context(tc.tile_pool(name="opp", bufs=2, space="PSUM"))

        def load(pool, dram_ap, dtype):
            t = pool.tile(list(dram_ap.shape), dtype, tag=dram_ap.tensor.name)
            nc.sync.dma_start(out=t, in_=dram_ap)
            return t

        wq_sb = load(singles, wq_d, BF16)    # [128, NCT, 512]  (c, kt, h*64+d)
        wk_sb = load(singles, wk_d, BF16)
        wv_sb = load(singles, wv_d, BF16)
        wp_sb = load(singles, wp_d, BF16)    # [128, NCT, 512]
        w1_sb = load(singles, w1_d, BF16)    # [128, NCT, 2048]
        w2_sb = load(singles, w2_d, BF16)    # [128, NF, 512]
        b1_sb = load(singles, b1_d, F32)     # [128, NF]
        bp_sb = load(singles, bp_d, BF16) if cfg["has_bp"] else None
        b2_sb = load(singles, b2_d, BF16) if cfg["has_b2"] else None
        cq_sb = load(singles, cq_d, BF16) if cfg["has_ln1b"] else None  # [3,512]
        trimask = load(singles, trimask_d, BF16)    # [128,128] keep t>=s
        ident = load(singles, ident_d, BF16)        # [128,128]
        ones_row = load(singles, ones_row_d, BF16)  # [1, 512]
        eps_t = singles.tile([P, 1], F32)
        nc.vector.memset(eps_t, EPS)

        def layernorm_T(x_tiles, hT_pool, n_tag):
            """x tiles [t,c] fp32 -> (x-mu)*rstd bf16 -> DMA-transpose to
            [c,t] tiles. Affine is folded into downstream weights."""
            n_tiles = []
            for t in range(NT):
                st6 = stat.tile([P, 6], F32, tag="st6")
                nc.vector.bn_stats(out=st6, in_=x_tiles[t])
                mv = stat.tile([P, 2], F32, tag="mv")
                nc.vector.bn_aggr(out=mv, in_=st6)
                lnv = stat.tile([P, 1], F32, tag="lnv")
                nc.scalar.activation(out=lnv, in_=mv[:, 1:2], func=AF.Ln,
                                     bias=eps_t)
                rstd = stat.tile([P, 1], F32, tag="rstd")
                nc.scalar.activation(out=rstd, in_=lnv, func=AF.Exp,
                                     scale=-0.5)
                n_t = nrm.tile([P, T], BF16, tag=n_tag)
                nc.vector.tensor_scalar(out=n_t, in0=x_tiles[t],
                                        scalar1=mv[:, 0:1], scalar2=rstd,
                                        op0=OP.subtract, op1=OP.mult)
                n_tiles.append(n_t)
            hT = []
            for i in range(NCT):
                tp = tpp.tile([P, T], BF16, tag="tps")
                for t in range(NT):
                    nc.tensor.transpose(tp[:, P * t:P * (t + 1)],
                                        n_tiles[t][:, P * i:P * (i + 1)],
                                        ident)
                h_i = hT_pool.tile([P, T], BF16)
                nc.vector.tensor_copy(out=h_i, in_=tp)
                hT.append(h_i)
            return hT

        # ---- LN1 for all batch items (overlaps initial weight DMA) ----
        hTs = []
        for b in range(NB):
            x_tiles = []
            for t in range(NT):
                x_t = xp.tile([P, C], F32)
                nc.gpsimd.dma_start(out=x_t, in_=x_d[b, P * t:P * (t + 1), :])
                x_tiles.append(x_t)
            hTs.append(layernorm_T(x_tiles, hTp, "n1"))

        h2Ts = []
        for b in range(NB):
            hT = hTs[b]
            # ---- QKV ----
            qT, kT = [], []
            for pr in range(NPAIR):
                sl = slice(P * pr, P * (pr + 1))
                qps = mmp.tile([P, T], F32, tag="mm")
                for kt in range(NCT):
                    nc.tensor.matmul(qps, wq_sb[:, kt, sl], hT[kt],
                                     start=(kt == 0),
                                     stop=(kt == NCT - 1
                                           and not cfg["has_ln1b"]),
                                     skip_group_check=True)
                if cfg["has_ln1b"]:
                    nc.tensor.matmul(qps, cq_sb[0:1, sl], ones_row,
                                     start=False, stop=True,
                                     skip_group_check=True)
                q_sb = qkp.tile([P, T], BF16, tag="qk")
                nc.scalar.activation(out=q_sb, in_=qps, func=AF.Copy)
                qT.append(q_sb)
                kps = mmp.tile([P, T], F32, tag="mm")
                for kt in range(NCT):
                    nc.tensor.matmul(kps, wk_sb[:, kt, sl], hT[kt],
                                     start=(kt == 0),
                                     stop=(kt == NCT - 1
                                           and not cfg["has_ln1b"]),
                                     skip_group_check=True)
                if cfg["has_ln1b"]:
                    nc.tensor.matmul(kps, cq_sb[1:2, sl], ones_row,
                                     start=False, stop=True,
                                     skip_group_check=True)
                k_sb = qkp.tile([P, T], BF16, tag="qk")
                nc.vector.tensor_copy(out=k_sb, in_=kps)
                kT.append(k_sb)
            v_aug = []
            for st in range(NT):
                sl = slice(P * st, P * (st + 1))
                vps = mmp.tile([P, C], F32, tag="mm")
                for kt in range(NCT):
                    nc.tensor.matmul(vps, hT[kt][:, sl], wv_sb[:, kt, :],
                                     start=(kt == 0),
                                     stop=(kt == NCT - 1
                                           and not cfg["has_ln1b"]),
                                     skip_group_check=True)
                if cfg["has_ln1b"]:
                    nc.tensor.matmul(vps, ones_row[:, 0:P], cq_sb[2:3, :],
                                     start=False, stop=True,
                                     skip_group_check=True)
                va = vp.tile([P, H, DA], BF16)
                nc.vector.memset(va[:, :, D:DA], 1.0)
                nc.vector.tensor_copy(
                    out=va[:, :, 0:D],
                    in_=vps[:].rearrange("p (h d) -> p h d", h=H))
                v_aug.append(va)

            # ---- scores^T + exp (per head, per s-tile) ----
            # expT[h][i] covers t in [P*i, T): tile [P, T - P*i]
            expT = [[None] * NT for _ in range(H)]
            for h in range(H):
                pr, off = divmod(h, 2)
                off *= D
                for i in range(NT):
                    w = T - P * i
                    sc = scp.tile([P, T], F32, tag="sc")
                    nc.tensor.matmul(sc[:, 0:w],
                                     kT[pr][off:off + D, P * i:P * (i + 1)],
                                     qT[pr][off:off + D, P * i:],
                                     start=True, stop=True)
                    e_t = expp.tile([P, w], BF16, tag=f"e{i}")
                    nc.scalar.activation(out=e_t, in_=sc[:, 0:w], func=AF.Exp)
                    # causal mask on the diagonal block (keep t >= s)
                    nc.vector.tensor_tensor(out=e_t[:, 0:P], in0=e_t[:, 0:P],
                                            in1=trimask, op=OP.mult)
                    expT[h][i] = e_t

            # ---- attention out + normalize (t-tile major, head pairs) ----
            o_sb = []
            for m in range(NT):
                o_t = osp.tile([P, C], BF16)
                for pr in range(NPAIR):
                    o2 = opp.tile([P, 2 * DA], F32, tag="op")
                    for h01 in range(2):
                        h = 2 * pr + h01
                        for i in range(m + 1):
                            lhs = expT[h][i][:, P * (m - i):P * (m - i + 1)]
                            nc.tensor.matmul(o2[:, DA * h01:DA * (h01 + 1)],
                                             lhs, v_aug[i][:, h, :],
                                             start=(i == 0), stop=(i == m),
                                             skip_group_check=True)
                    l_ap = bass.AP(tensor=o2[:].tensor,
                                   offset=o2[:, D:D + 1].offset,
                                   ap=[o2[:].ap[0], [DA, 2]])
                    linv = stat.tile([P, 2], F32, tag="linv")
                    nc.vector.reciprocal(out=linv, in_=l_ap)
                    o_part = bass.AP(tensor=o2[:].tensor, offset=o2[:].offset,
                                     ap=[o2[:].ap[0], [DA, 2], [1, D]])
                    out3 = o_t[:, P * pr:P * (pr + 1)].rearrange(
                        "p (a d) -> p a d", a=2)
                    nc.vector.tensor_tensor(out=out3, in0=o_part,
                                            in1=_bcast_free(linv[:], D),
                                            op=OP.mult)
                o_sb.append(o_t)

            # ---- transpose o (DMA xbar) ----
            oT = []
            for i in range(NCT):
                tp = tpp.tile([P, T], BF16, tag="tps")
                for m in range(NT):
                    nc.tensor.transpose(tp[:, P * m:P * (m + 1)],
                                        o_sb[m][:, P * i:P * (i + 1)],
                                        ident)
                oT_i = oTp.tile([P, T], BF16)
                nc.vector.tensor_copy(out=oT_i, in_=tp)
                oT.append(oT_i)

            # ---- proj + residual; spill x2 to DRAM scratch ----
            x2_tiles = []
            for m in range(NT):
                yps = mmp.tile([P, C], F32, tag="mm")
                for kt in range(NCT):
                    nc.tensor.matmul(yps, oT[kt][:, P * m:P * (m + 1)],
                                     wp_sb[:, kt, :], start=(kt == 0),
                                     stop=(kt == NCT - 1
                                           and not cfg["has_bp"]),
                                     skip_group_check=True)
                if cfg["has_bp"]:
                    nc.tensor.matmul(yps, ones_row[:, 0:P], bp_sb,
                                     start=False, stop=True,
                                     skip_group_check=True)
                x_t = xp.tile([P, C], F32)
                nc.gpsimd.dma_start(out=x_t, in_=x_d[b, P * m:P * (m + 1), :])
                x2_t = x2p.tile([P, C], F32)
                nc.vector.tensor_tensor(out=x2_t, in0=yps, in1=x_t, op=OP.add)
                nc.sync.dma_start(out=x2s_d[b, P * m:P * (m + 1), :], in_=x2_t)
                x2_tiles.append(x2_t)

            # ---- LN2 (affine folded into w1/b1) ----
            h2Ts.append(layernorm_T(x2_tiles, h2Tp, "n2"))

        # ---- FFN for all batch items ----
        for b in range(NB):
            h2T = h2Ts[b]
            zT = []
            for j in range(NF):
                zps = mmp.tile([P, T], F32, tag="mm")
                for kt in range(NCT):
                    nc.tensor.matmul(zps, w1_sb[:, kt, P * j:P * (j + 1)],
                                     h2T[kt], start=(kt == 0),
                                     stop=(kt == NCT - 1))
                z_j = zp.tile([P, T], BF16)
                nc.scalar.activation(out=z_j, in_=zps, func=AF.Relu,
                                     bias=b1_sb[:, j:j + 1])
                zT.append(z_j)
            for m in range(NT):
                fps = mmp.tile([P, C], F32, tag="mm")
                for kt in range(NF):
                    nc.tensor.matmul(fps, zT[kt][:, P * m:P * (m + 1)],
                                     w2_sb[:, kt, :], start=(kt == 0),
                                     stop=(kt == NF - 1
                                           and not cfg["has_b2"]),
                                     skip_group_check=True)
                if cfg["has_b2"]:
                    nc.tensor.matmul(fps, ones_row[:, 0:P], b2_sb,
                                     start=False, stop=True,
                                     skip_group_check=True)
                x2r = x2rp.tile([P, C], F32)
                nc.gpsimd.dma_start(out=x2r,
                                    in_=x2s_d[b, P * m:P * (m + 1), :])
                o_t = outp.tile([P, C], F32)
                nc.vector.tensor_tensor(out=o_t, in0=fps, in1=x2r, op=OP.add)
                nc.gpsimd.dma_start(out=out_d[b, P * m:P * (m + 1), :],
                                    in_=o_t)


def _build(cfg):
    nc = bacc.Bacc("TRN2", target_bir_lowering=False, debug=False,
                   num_devices=NCORES)
    d = nc.dram_tensor
    io = (
        d("x", [NB, T, C], F32, kind="ExternalInput").ap(),
        d("wq", [P, NCT, C], BF16, kind="ExternalInput").ap(),
        d("wk", [P, NCT, C], BF16, kind="ExternalInput").ap(),
        d("wv", [P, NCT, C], BF16, kind="ExternalInput").ap(),
        d("wp", [P, NCT, C], BF16, kind="ExternalInput").ap(),
        d("w1", [P, NCT, FF], BF16, kind="ExternalInput").ap(),
        d("w2", [P, NF, C], BF16, kind="ExternalInput").ap(),
        d("b1", [P, NF], F32, kind="ExternalInput").ap(),
        d("bp", [1, C], BF16, kind="ExternalInput").ap(),
        d("b2", [1, C], BF16, kind="ExternalInput").ap(),
        d("cq", [3, C], BF16, kind="ExternalInput").ap(),
        d("trimask", [P, P], BF16, kind="ExternalInput").ap(),
        d("ident", [P, P], BF16, kind="ExternalInput").ap(),
        d("ones_row", [1, C], BF16, kind="ExternalInput").ap(),
        d("x2s", [NB, T, C], F32, kind="Internal").ap(),
        d("out", [NB, T, C], F32, kind="ExternalOutput").ap(),
    )
    with tile.TileContext(nc) as tc:
        _body(tc, io, cfg)
    nc.compile()
    return nc


def _ktile(w, part):
    """[K, M] -> [128, K//128, M] with K = 128*kt + p."""
    k, m = w.shape
    return np.ascontiguousarray(
        w.reshape(k // part, part, m).transpose(1, 0, 2))


def _col(v, part):
    """[N] -> [128, N//128] with n = 128*j + p."""
    return np.ascontiguousarray(v.reshape(-1, part).T)


def kernel(**inputs):
    f32 = lambda a: np.asarray(a, np.float32)
    x = f32(inputs["x"])
    wq = f32(inputs["wq"]).transpose(1, 0, 2).reshape(C, C)   # [c, h*D+d]
    wk = f32(inputs["wk"]).transpose(1, 0, 2).reshape(C, C)
    wv = f32(inputs["wv"]).transpose(1, 0, 2).reshape(C, C)
    w1 = f32(inputs["w1"])
    g1 = f32(inputs["ln1_g"])[:, None]
    b1ln = f32(inputs["ln1_b"])
    g2 = f32(inputs["ln2_g"])[:, None]
    b2ln = f32(inputs["ln2_b"])

    cfg = {
        "has_ln1b": bool(np.any(b1ln != 0.0)),
        "has_bp": bool(np.any(f32(inputs["b_proj"]) != 0.0)),
        "has_b2": bool(np.any(f32(inputs["b2"]) != 0.0)),
    }
    key = tuple(sorted(cfg.items()))
    if key not in _CACHE:
        _CACHE[key] = _build(cfg)
    nc = _CACHE[key]

    # fold LN affines (and the score scale) into the weights
    wq_f = (g1 * wq) * SCALE
    wk_f = g1 * wk
    wv_f = g1 * wv
    w1_f = g2 * w1
    b1_f = b2ln @ w1 + f32(inputs["b1"])
    cq = np.stack([(b1ln @ wq) * SCALE, b1ln @ wk, b1ln @ wv])  # [3, C]

    common = {
        "wq": _ktile(wq_f, P).astype(bf16),
        "wk": _ktile(wk_f, P).astype(bf16),
        "wv": _ktile(wv_f, P).astype(bf16),
        "wp": _ktile(f32(inputs["w_proj"]), P).astype(bf16),
        "w1": _ktile(w1_f, P).astype(bf16),
        "w2": _ktile(f32(inputs["w2"]), P).astype(bf16),
        "b1": _col(b1_f, P),
        "bp": f32(inputs["b_proj"]).reshape(1, C).astype(bf16),
        "b2": f32(inputs["b2"]).reshape(1, C).astype(bf16),
        "cq": cq.astype(bf16),
        "trimask": np.triu(np.ones((P, P), np.float32)).astype(bf16),
        "ident": np.eye(P, dtype=bf16),
        "ones_row": np.ones((1, C), bf16),
    }
    in_maps = [dict(common, x=np.ascontiguousarray(x[c * NB:(c + 1) * NB]))
               for c in range(NCORES)]

    res = bass_utils.run_bass_kernel_spmd(nc, in_maps,
                                          core_ids=list(range(NCORES)),
                                          trace=_CACHE.get("trace", False))
    _CACHE["last_result"] = res
    return np.concatenate([r["out"] for r in res.results], axis=0)


# revision 14
# speedup vs baseline: 1.5505x; 1.1115x over previous
"""Trainium2 Bass kernel for a dense transformer block.

Reference computation (per batch item, fp32 inputs):
    h   = LN(x; ln1_g, ln1_b)
    q,k,v = per-head projections of h        (H=8 heads, D=64)
    scores = (q @ k^T) * C**-0.5, causal-masked, softmax
    o   = scores @ v, heads concatenated
    x2  = x + o @ w_proj + b_proj
    out = x2 + relu(LN(x2; ln2_g, ln2_b) @ w1 + b1) @ w2 + b2

Sharding: pure data parallel over batch. B=32 across 8 cores -> 4 batch
items per core, weights replicated, no collectives.

Per-core design notes:
  - LN affine transforms are folded into the following matmul weights on
    the host: wq/wk/wv absorb diag(ln1_g) (and wq also the C**-0.5 score
    scale); w1 absorbs diag(ln2_g) and b1 absorbs ln2_b @ w1. When the
    LN bias is nonzero for the QKV path, rank-1 correction matmuls are
    emitted (skipped for zero bias).
  - LN stats run in [t, c] layout (free-dim bn_stats); the normalized
    bf16 output is transposed to [c, t] by DMA-xbar transposes, feeding
    all matmuls with contraction over c. rstd = Exp(-0.5*Log(var+eps))
    so the whole kernel uses one ACT table set (exp/log/relu/copy).
  - Scores are computed transposed per head: scoresT[s, t] with
    lhsT=k^T slice / rhs=q^T (head-pair packed [128, T]); exp'd scores
    (bf16) are then directly the lhsT of the attn@v matmuls. The causal
    mask multiplies the diagonal 128x128 block after exp (DVE, bf16).
  - v is stored interleaved [128, 8, 65] with a ones column per head, so
    each attn@v matmul (N=65) also produces the softmax denominator in
    its last column; two heads share one PSUM bank [128, 130]. The
    normalize step is one fused tensor_tensor multiply with a step-0
    broadcast AP of the reciprocal denominators.
  - FFN hidden z is computed transposed [f, t] so relu+b1 is one ACT op
    with per-partition bias and z^T directly feeds FFN2 as lhsT.
  - x2 (attention residual) is spilled to a DRAM scratch tensor and
    reloaded for the final residual add to keep SBUF under budget.
  - Phase order: LN1 for all batch items up front (overlaps the initial
    weight DMA); per-item QKV->scores/exp->attn@v->proj->LN2; FFN for
    all items at the end. This gives the PE filler work across batch
    items during LN dependency chains.

All matmuls run in bf16 (fp32 PSUM accumulation).
"""

import contextlib

import numpy as np
import ml_dtypes

import concourse.bass as bass
import concourse.bacc as bacc
import concourse.tile as tile
import concourse.mybir as mybir
from concourse import bass_utils

B, T, C, H, D = 32, 512, 512, 8, 64
NCORES = 8
NB = B // NCORES          # batch items per core
P = 128
NT = T // P               # 4 token tiles
NCT = C // P              # 4 channel tiles
FF = 4 * C                # 2048
NF = FF // P              # 16 hidden tiles
EPS = 1e-5
SCALE = float(C) ** -0.5
NPAIR = H // 2            # head pairs (2 heads x 64 = 128 partitions)
DA = D + 1                # v columns per head incl. ones column

F32 = mybir.dt.float32
BF16 = mybir.dt.bfloat16
AF = mybir.ActivationFunctionType
OP = mybir.AluOpType
bf16 = ml_dtypes.bfloat16

_CACHE = {}


def _bcast_free(ap, reps):
    """Append a step-0 innermost dim: each free element read `reps` times."""
    return bass.AP(tensor=ap.tensor, offset=ap.offset, ap=[*ap.ap, [0, reps]])


def _body(tc, io, cfg):
    nc = tc.nc
    (x_d, wq_d, wk_d, wv_d, wp_d, w1_d, w2_d, b1_d, bp_d, b2_d, cq_d,
     trimask_d, ident_d, ones_row_d, x2s_d, out_d) = io

    ctx = contextlib.ExitStack()
    with ctx:
        singles = ctx.enter_context(tc.tile_pool(name="singles", bufs=1))
        xp = ctx.enter_context(tc.tile_pool(name="xp", bufs=6))
        x2p = ctx.enter_context(tc.tile_pool(name="x2p", bufs=6))
        x2rp = ctx.enter_context(tc.tile_pool(name="x2rp", bufs=4))
        nrm = ctx.enter_context(tc.tile_pool(name="nrm", bufs=6))
        stat = ctx.enter_context(tc.tile_pool(name="stat", bufs=12))
        hTp = ctx.enter_context(tc.tile_pool(name="hTp", bufs=NB * NCT))
        qkp = ctx.enter_context(tc.tile_pool(name="qkp", bufs=2 * NPAIR + 2))
        vp = ctx.enter_context(tc.tile_pool(name="vp", bufs=NT + 2))
        expp = ctx.enter_context(tc.tile_pool(name="expp", bufs=H + 2))
        osp = ctx.enter_context(tc.tile_pool(name="osp", bufs=NT + 1))
        oTp = ctx.enter_context(tc.tile_pool(name="oTp", bufs=NCT + 1))
        h2Tp = ctx.enter_context(tc.tile_pool(name="h2Tp", bufs=NB * NCT))
        zp = ctx.enter_context(tc.tile_pool(name="zp", bufs=NF + 1))
        outp = ctx.enter_context(tc.tile_pool(name="outp", bufs=4))
        # PSUM: 8 banks total
        mmp = ctx.enter_context(tc.tile_pool(name="mmp", bufs=3, space="PSUM"))
        tpp = ctx.enter_context(tc.tile_pool(name="tpp", bufs=1, space="PSUM"))
        scp = ctx.enter_context(tc.tile_pool(name="scp", bufs=2, space="PSUM"))
        opp = ctx.enter_context(tc.tile_pool(name="opp", bufs=2, space="PSUM"))

        def load(pool, dram_ap, dtype):
            t = pool.tile(list(dram_ap.shape), dtype, tag=dram_ap.tensor.name)
            nc.sync.dma_start(out=t, in_=dram_ap)
            return t

        # attention-path weights first; FFN weights are loaded after the
        # LN1 phase is emitted so their DMA doesn't delay the start
        wq_sb = load(singles, wq_d, BF16)    # [128, NCT, 512]  (c, kt, h*64+d)
        wk_sb = load(singles, wk_d, BF16)
        wv_sb = load(singles, wv_d, BF16)
        wp_sb = load(singles, wp_d, BF16)    # [128, NCT, 512]
        bp_sb = load(singles, bp_d, BF16) if cfg["has_bp"] else None
        cq_sb = load(singles, cq_d, BF16) if cfg["has_ln1b"] else None  # [3,512]
        trimask = load(singles, trimask_d, BF16)    # [128,128] keep t>=s
        ident = load(singles, ident_d, BF16)        # [128,128]
        ones_row = load(singles, ones_row_d, BF16)  # [1, 512]
        eps_t = singles.tile([P, 1], F32)
        nc.vector.memset(eps_t, EPS)

        def ln_stats(x_tiles, mv_all, base):
            """bn stats for NT tiles into mv_all columns [2b, 2b+1]."""
            for t in range(NT):
                st6 = stat.tile([P, 6], F32, tag="st6")
                nc.vector.bn_stats(out=st6, in_=x_tiles[t])
                i = base + t
                nc.vector.bn_aggr(out=mv_all[:, 2 * i:2 * i + 2], in_=st6)

        def ln_rstd(mv_all, rstd_all, n):
            """rstd = Exp(-0.5 * Ln(var + eps)), batched over n columns so
            the Ln/Exp table sets load once per batch, not per tile."""
            var_ap = bass.AP(tensor=mv_all[:].tensor,
                             offset=mv_all[:, 1:2].offset,
                             ap=[mv_all[:].ap[0], [2, n]])
            lnv = stat.tile([P, n], F32, tag="lnv")
            nc.scalar.activation(out=lnv, in_=var_ap, func=AF.Ln, bias=eps_t)
            nc.scalar.activation(out=rstd_all, in_=lnv, func=AF.Exp,
                                 scale=-0.5)

        def ln_apply_T(get_x, mv_all, rstd_all, base, hT_pool, n_tag):
            """(x-mu)*rstd bf16 -> PE transpose -> [c,t] tiles."""
            n_tiles = []
            for t in range(NT):
                i = base + t
                n_t = nrm.tile([P, T], BF16, tag=n_tag)
                nc.vector.tensor_scalar(out=n_t, in0=get_x(t),
                                        scalar1=mv_all[:, 2 * i:2 * i + 1],
                                        scalar2=rstd_all[:, i:i + 1],
                                        op0=OP.subtract, op1=OP.mult)
                n_tiles.append(n_t)
            hT = []
            for i in range(NCT):
                tp = tpp.tile([P, T], BF16, tag="tps")
                for t in range(NT):
                    nc.tensor.transpose(tp[:, P * t:P * (t + 1)],
                                        n_tiles[t][:, P * i:P * (i + 1)],
                                        ident)
                h_i = hT_pool.tile([P, T], BF16)
                nc.vector.tensor_copy(out=h_i, in_=tp)
                hT.append(h_i)
            return hT

        # ---- LN1 for all batch items (overlaps initial weight DMA) ----
        mv1 = singles.tile([P, 2 * NB * NT], F32, tag="mv1")
        rstd1 = singles.tile([P, NB * NT], F32, tag="rstd1")
        def load_x(b, t):
            x_t = xp.tile([P, C], F32)
            nc.gpsimd.dma_start(out=x_t, in_=x_d[b, P * t:P * (t + 1), :])
            return x_t

        for b in range(NB):
            ln_stats([load_x(b, t) for t in range(NT)], mv1, NT * b)
        ln_rstd(mv1, rstd1, NB * NT)
        # x tiles are reloaded for the normalize pass so the stats pass
        # doesn't pin 16 pool slots across the whole batched-rstd barrier
        hTs = [ln_apply_T(lambda t, b=b: load_x(b, t), mv1, rstd1, NT * b,
                          hTp, "n1")
               for b in range(NB)]

        # FFN weights load now (needed only in the tail phase)
        w1_sb = load(singles, w1_d, BF16)    # [128, NCT, 2048]
        w2_sb = load(singles, w2_d, BF16)    # [128, NF, 512]
        b1_sb = load(singles, b1_d, F32)     # [128, NF]

        def emit_qkv(hT):
            qT, kT = [], []
            for pr in range(NPAIR):
                sl = slice(P * pr, P * (pr + 1))
                qps = mmp.tile([P, T], F32, tag="mm")
                for kt in range(NCT):
                    nc.tensor.matmul(qps, wq_sb[:, kt, sl], hT[kt],
                                     start=(kt == 0),
                                     stop=(kt == NCT - 1
                                           and not cfg["has_ln1b"]),
                                     skip_group_check=True)
                if cfg["has_ln1b"]:
                    nc.tensor.matmul(qps, cq_sb[0:1, sl], ones_row,
                                     start=False, stop=True,
                                     skip_group_check=True)
                q_sb = qkp.tile([P, T], BF16, tag="qk")
                nc.scalar.activation(out=q_sb, in_=qps, func=AF.Copy)
                qT.append(q_sb)
                kps = mmp.tile([P, T], F32, tag="mm")
                for kt in range(NCT):
                    nc.tensor.matmul(kps, wk_sb[:, kt, sl], hT[kt],
                                     start=(kt == 0),
                                     stop=(kt == NCT - 1
                                           and not cfg["has_ln1b"]),
                                     skip_group_check=True)
                if cfg["has_ln1b"]:
                    nc.tensor.matmul(kps, cq_sb[1:2, sl], ones_row,
                                     start=False, stop=True,
                                     skip_group_check=True)
                k_sb = qkp.tile([P, T], BF16, tag="qk")
                nc.vector.tensor_copy(out=k_sb, in_=kps)
                kT.append(k_sb)
            v_aug = []
            for st in range(NT):
                sl = slice(P * st, P * (st + 1))
                vps = mmp.tile([P, C], F32, tag="mm")
                for kt in range(NCT):
                    nc.tensor.matmul(vps, hT[kt][:, sl], wv_sb[:, kt, :],
                                     start=(kt == 0),
                                     stop=(kt == NCT - 1
                                           and not cfg["has_ln1b"]),
                                     skip_group_check=True)
                if cfg["has_ln1b"]:
                    nc.tensor.matmul(vps, ones_row[:, 0:P], cq_sb[2:3, :],
                                     start=False, stop=True,
                                     skip_group_check=True)
                va = vp.tile([P, H, DA], BF16)
                nc.vector.memset(va[:, :, D:DA], 1.0)
                nc.vector.tensor_copy(
                    out=va[:, :, 0:D],
                    in_=vps[:].rearrange("p (h d) -> p h d", h=H))
                v_aug.append(va)
            return qT, kT, v_aug

        # software-pipelined emission: QKV(b+1) is emitted before LN2(b)
        # so the PE has ready matmul work during LN2's DVE/ACT chain
        qkv_next = emit_qkv(hTs[0])
        h2Ts = []
        for b in range(NB):
            hT = hTs[b]
            qT, kT, v_aug = qkv_next

            # ---- scores^T + exp (per head, per s-tile) ----
            # expT[h][i] covers t in [P*i, T): tile [P, T - P*i]
            expT = [[None] * NT for _ in range(H)]
            for h in range(H):
                pr, off = divmod(h, 2)
                off *= D
                for i in range(NT):
                    w = T - P * i
                    sc = scp.tile([P, T], F32, tag="sc")
                    nc.tensor.matmul(sc[:, 0:w],
                                     kT[pr][off:off + D, P * i:P * (i + 1)],
                                     qT[pr][off:off + D, P * i:],
                                     start=True, stop=True)
                    e_t = expp.tile([P, w], BF16, tag=f"e{i}")
                    nc.scalar.activation(out=e_t, in_=sc[:, 0:w], func=AF.Exp)
                    # causal mask on the diagonal block (keep t >= s)
                    nc.vector.tensor_tensor(out=e_t[:, 0:P], in0=e_t[:, 0:P],
                                            in1=trimask, op=OP.mult)
                    expT[h][i] = e_t

            # ---- attention out + normalize (t-tile major, head pairs) ----
            o_sb = []
            for m in range(NT):
                o_t = osp.tile([P, C], BF16)
                for pr in range(NPAIR):
                    o2 = opp.tile([P, 2 * DA], F32, tag="op")
                    for h01 in range(2):
                        h = 2 * pr + h01
                        for i in range(m + 1):
                            lhs = expT[h][i][:, P * (m - i):P * (m - i + 1)]
                            nc.tensor.matmul(o2[:, DA * h01:DA * (h01 + 1)],
                                             lhs, v_aug[i][:, h, :],
                                             start=(i == 0), stop=(i == m),
                                             skip_group_check=True)
                    l_ap = bass.AP(tensor=o2[:].tensor,
                                   offset=o2[:, D:D + 1].offset,
                                   ap=[o2[:].ap[0], [DA, 2]])
                    linv = stat.tile([P, 2], F32, tag="linv")
                    nc.vector.reciprocal(out=linv, in_=l_ap)
                    o_part = bass.AP(tensor=o2[:].tensor, offset=o2[:].offset,
                                     ap=[o2[:].ap[0], [DA, 2], [1, D]])
                    out3 = o_t[:, P * pr:P * (pr + 1)].rearrange(
                        "p (a d) -> p a d", a=2)
                    nc.vector.tensor_tensor(out=out3, in0=o_part,
                                            in1=_bcast_free(linv[:], D),
                                            op=OP.mult)
                o_sb.append(o_t)

            # ---- transpose o (DMA xbar) ----
            oT = []
            for i in range(NCT):
                tp = tpp.tile([P, T], BF16, tag="tps")
                for m in range(NT):
                    nc.tensor.transpose(tp[:, P * m:P * (m + 1)],
                                        o_sb[m][:, P * i:P * (i + 1)],
                                        ident)
                oT_i = oTp.tile([P, T], BF16)
                nc.vector.tensor_copy(out=oT_i, in_=tp)
                oT.append(oT_i)

            # ---- proj + residual; spill x2 to DRAM scratch ----
            x2_tiles = []
            for m in range(NT):
                yps = mmp.tile([P, C], F32, tag="mm")
                for kt in range(NCT):
                    nc.tensor.matmul(yps, oT[kt][:, P * m:P * (m + 1)],
                                     wp_sb[:, kt, :], start=(kt == 0),
                                     stop=(kt == NCT - 1
                                           and not cfg["has_bp"]),
                                     skip_group_check=True)
                if cfg["has_bp"]:
                    nc.tensor.matmul(yps, ones_row[:, 0:P], bp_sb,
                                     start=False, stop=True,
                                     skip_group_check=True)
                x_t = xp.tile([P, C], F32)
                nc.gpsimd.dma_start(out=x_t, in_=x_d[b, P * m:P * (m + 1), :])
                x2_t = x2p.tile([P, C], F32)
                nc.vector.tensor_tensor(out=x2_t, in0=yps, in1=x_t, op=OP.add)
                nc.sync.dma_start(out=x2s_d[b, P * m:P * (m + 1), :], in_=x2_t)
                x2_tiles.append(x2_t)

            # QKV of the next batch item keeps the PE busy during LN2
            if b + 1 < NB:
                qkv_next = emit_qkv(hTs[b + 1])

            # ---- LN2 (affine folded into w1/b1) ----
            mv2 = stat.tile([P, 2 * NT], F32, tag="mv2")
            rstd2 = stat.tile([P, NT], F32, tag="rstd2")
            ln_stats(x2_tiles, mv2, 0)
            ln_rstd(mv2, rstd2, NT)
            h2Ts.append(ln_apply_T(lambda t: x2_tiles[t], mv2, rstd2, 0,
                                   h2Tp, "n2"))

        # ---- FFN for all batch items ----
        for b in range(NB):
            h2T = h2Ts[b]
            zT = []
            for j in range(NF):
                zps = mmp.tile([P, T], F32, tag="mm")
                for kt in range(NCT):
                    nc.tensor.matmul(zps, w1_sb[:, kt, P * j:P * (j + 1)],
                                     h2T[kt], start=(kt == 0),
                                     stop=(kt == NCT - 1))
                z_j = zp.tile([P, T], BF16)
                nc.scalar.activation(out=z_j, in_=zps, func=AF.Relu,
                                     bias=b1_sb[:, j:j + 1])
                zT.append(z_j)
            for m in range(NT):
                fps = mmp.tile([P, C], F32, tag="mm")
                for kt in range(NF):
                    nc.tensor.matmul(fps, zT[kt][:, P * m:P * (m + 1)],
                                     w2_sb[:, kt, :], start=(kt == 0),
                                     stop=(kt == NF - 1
                                           and not cfg["has_b2"]),
                                     skip_group_check=True)
                if cfg["has_b2"]:
                    nc.tensor.matmul(fps, ones_row[:, 0:P], b2_sb,
                                     start=False, stop=True,
                                     skip_group_check=True)
                x2r = x2rp.tile([P, C], F32)
                nc.gpsimd.dma_start(out=x2r,
                                    in_=x2s_d[b, P * m:P * (m + 1), :])
                o_t = outp.tile([P, C], F32)
                nc.vector.tensor_tensor(out=o_t, in0=fps, in1=x2r, op=OP.add)
                nc.gpsimd.dma_start(out=out_d[b, P * m:P * (m + 1), :],
                                    in_=o_t)


def _build(cfg):
    nc = bacc.Bacc("TRN2", target_bir_lowering=False, debug=False,
                   num_devices=NCORES)
    d = nc.dram_tensor
    io = (
        d("x", [NB, T, C], F32, kind="ExternalInput").ap(),
        d("wq", [P, NCT, C], BF16, kind="ExternalInput").ap(),
        d("wk", [P, NCT, C], BF16, kind="ExternalInput").ap(),
        d("wv", [P, NCT, C], BF16, kind="ExternalInput").ap(),
        d("wp", [P, NCT, C], BF16, kind="ExternalInput").ap(),
        d("w1", [P, NCT, FF], BF16, kind="ExternalInput").ap(),
        d("w2", [P, NF, C], BF16, kind="ExternalInput").ap(),
        d("b1", [P, NF], F32, kind="ExternalInput").ap(),
        d("bp", [1, C], BF16, kind="ExternalInput").ap(),
        d("b2", [1, C], BF16, kind="ExternalInput").ap(),
        d("cq", [3, C], BF16, kind="ExternalInput").ap(),
        d("trimask", [P, P], BF16, kind="ExternalInput").ap(),
        d("ident", [P, P], BF16, kind="ExternalInput").ap(),
        d("ones_row", [1, C], BF16, kind="ExternalInput").ap(),
        d("x2s", [NB, T, C], F32, kind="Internal").ap(),
        d("out", [NB, T, C], F32, kind="ExternalOutput").ap(),
    )
    with tile.TileContext(nc) as tc:
        _body(tc, io, cfg)
    nc.compile()
    return nc


def _ktile(w, part):
    """[K, M] -> [128, K//128, M] with K = 128*kt + p."""
    k, m = w.shape
    return np.ascontiguousarray(
        w.reshape(k // part, part, m).transpose(1, 0, 2))


def _col(v, part):
    """[N] -> [128, N//128] with n = 128*j + p."""
    return np.ascontiguousarray(v.reshape(-1, part).T)


def kernel(**inputs):
    f32 = lambda a: np.asarray(a, np.float32)
    x = f32(inputs["x"])
    wq = f32(inputs["wq"]).transpose(1, 0, 2).reshape(C, C)   # [c, h*D+d]
    wk = f32(inputs["wk"]).transpose(1, 0, 2).reshape(C, C)
    wv = f32(inputs["wv"]).transpose(1, 0, 2).reshape(C, C)
    w1 = f32(inputs["w1"])
    g1 = f32(inputs["ln1_g"])[:, None]
    b1ln = f32(inputs["ln1_b"])
    g2 = f32(inputs["ln2_g"])[:, None]
    b2ln = f32(inputs["ln2_b"])

    cfg = {
        "has_ln1b": bool(np.any(b1ln != 0.0)),
        "has_bp": bool(np.any(f32(inputs["b_proj"]) != 0.0)),
        "has_b2": bool(np.any(f32(inputs["b2"]) != 0.0)),
    }
    key = tuple(sorted(cfg.items()))
    if key not in _CACHE:
        _CACHE[key] = _build(cfg)
    nc = _CACHE[key]

    # fold LN affines (and the score scale) into the weights
    wq_f = (g1 * wq) * SCALE
    wk_f = g1 * wk
    wv_f = g1 * wv
    w1_f = g2 * w1
    b1_f = b2ln @ w1 + f32(inputs["b1"])
    cq = np.stack([(b1ln @ wq) * SCALE, b1ln @ wk, b1ln @ wv])  # [3, C]

    common = {
        "wq": _ktile(wq_f, P).astype(bf16),
        "wk": _ktile(wk_f, P).astype(bf16),
        "wv": _ktile(wv_f, P).astype(bf16),
        "wp": _ktile(f32(inputs["w_proj"]), P).astype(bf16),
        "w1": _ktile(w1_f, P).astype(bf16),
        "w2": _ktile(f32(inputs["w2"]), P).astype(bf16),
        "b1": _col(b1_f, P),
        "bp": f32(inputs["b_proj"]).reshape(1, C).astype(bf16),
        "b2": f32(inputs["b2"]).reshape(1, C).astype(bf16),
        "cq": cq.astype(bf16),
        "trimask": np.triu(np.ones((P, P), np.float32)).astype(bf16),
        "ident": np.eye(P, dtype=bf16),
        "ones_row": np.ones((1, C), bf16),
    }
    in_maps = [dict(common, x=np.ascontiguousarray(x[c * NB:(c + 1) * NB]))
               for c in range(NCORES)]

    res = bass_utils.run_bass_kernel_spmd(nc, in_maps,
                                          core_ids=list(range(NCORES)),
                                          trace=_CACHE.get("trace", False))
    _CACHE["last_result"] = res
    return np.concatenate([r["out"] for r in res.results], axis=0)
